# revision 22
# baseline (speedup 1.0000x reference)
"""Trainium2 Bass kernel for nn_DKOKernel (dense pairwise MLP + PSD head).

Math (per batch b, one NeuronCore per batch element):
  hx[f,i] = wx x_i;  hy[f,j] = wy y_j   (BN1 folded into wx/wy/c1)
  h1 = relu(hx_i + hy_j + c1)           (512)
  h2 = relu(W2 h1 + c2)                 (256)
  h3 = relu(W3 h2 + c3)                 (128)
  e  = W4 h3 + b4                       (64)
  s_i = sum_j e_ij;  out[i,j] = e_ij . s_i

Head algebra (never materializes e or s):
  q_i = sum_j h3_ij
  v_i = (W4^T W4) q_i + ny (W4^T b4) = M q_i + ny wc
  out[i,j] = h3_ij . v_i + c_i,  c_i = wc . q_i + ny|b4|^2
The +c_i term is applied on the HOST (q is shipped back), so the device
only computes the h3.v dot.

All matmuls run in bf16 (1 cycle/row); fp8 was measured too inaccurate
for the 2e-2 gate (h1-only fp8 already gives 1.7e-2).

Layout: features on partitions, (i-block, j) pairs on the free dim.
C=4 i-rows per chunk -> free dim 512 (fp32 PSUM bank max). Chunks are
processed in groups of G=4; L1 is produced per-group with two big DVE
ops per fc (broadcast add + relu) instead of 16 small fused ops.
"""

import os
import numpy as np
import ml_dtypes
from contextlib import ExitStack

import concourse.bacc as bacc
import concourse.tile as tile
from concourse import mybir
from concourse.bass_utils import run_bass_kernel_spmd

F32 = mybir.dt.float32
F32R = mybir.dt.float32r
BF16 = mybir.dt.bfloat16
AF = mybir.ActivationFunctionType
ALU = mybir.AluOpType
AX = mybir.AxisListType

EPS = 1e-5
B = 8
N = 128          # nx == ny
F = 128          # input feature dim
D1, D2, D3 = 512, 256, 128
C = 4            # i-rows per chunk -> 512 pairs per chunk
NCH = N // C     # 32 chunks
G = int(os.environ.get('GROUP', '4'))    # chunks per group
NG = NCH // G

NPBF = ml_dtypes.bfloat16


def build_module():
    nc = bacc.Bacc()

    xT = nc.declare_dram_parameter("xT", [F, N], BF16, isOutput=False)
    yT = nc.declare_dram_parameter("yT", [F, N], BF16, isOutput=False)
    wxT = nc.declare_dram_parameter("wxT", [F, D1], BF16, isOutput=False)
    wyT = nc.declare_dram_parameter("wyT", [F, D1], BF16, isOutput=False)
    w2d = nc.declare_dram_parameter("w2b", [128, 4, D2], BF16, isOutput=False)
    w3d = nc.declare_dram_parameter("w3b", [128, 2, D3], BF16, isOutput=False)
    Md = nc.declare_dram_parameter("M", [128, 128], F32, isOutput=False)
    Nwcd = nc.declare_dram_parameter("Nwc", [128], F32, isOutput=False)
    c1d = nc.declare_dram_parameter("c1", [4, 128], F32, isOutput=False)
    c2d = nc.declare_dram_parameter("c2", [2, 128], F32, isOutput=False)
    c3d = nc.declare_dram_parameter("c3", [128], F32, isOutput=False)
    out_d = nc.declare_dram_parameter("out", [N, N], BF16, isOutput=True)
    q_d = nc.declare_dram_parameter("q", [NG, 128, G * C], F32, isOutput=True)

    with tile.TileContext(nc) as tc:
        with ExitStack() as ctx:
            singles = ctx.enter_context(tc.tile_pool(name="singles", bufs=1))

            xT_s = singles.tile([F, N], BF16)
            yT_s = singles.tile([F, N], BF16)
            wxT_s = singles.tile([F, D1], BF16)
            wyT_s = singles.tile([F, D1], BF16)
            w2_s = singles.tile([128, 4, D2], BF16)
            w3_s = singles.tile([128, 2, D3], BF16)
            M_f = singles.tile([128, 128], F32)
            M_s = singles.tile([128, 128], F32R)
            Nwc_s = singles.tile([128, 1], F32)
            c1_s = singles.tile([128, 4], F32)
            c2_s = singles.tile([128, 2], F32)
            c3_s = singles.tile([128, 1], F32)
            hx_s = singles.tile([128, 4, N], BF16)
            hy_s = singles.tile([128, 4, N], BF16)
            stage = singles.tile([1, NCH, C * N], BF16)
            h1p = singles.tile([128, 4, G, C * N], BF16)
            hy_rep = singles.tile([128, 4, N, C], BF16)

            nc.sync.dma_start(out=xT_s, in_=xT[:, :])
            nc.sync.dma_start(out=yT_s, in_=yT[:, :])
            nc.sync.dma_start(out=wxT_s, in_=wxT[:, :])
            nc.sync.dma_start(out=wyT_s, in_=wyT[:, :])
            nc.sync.dma_start(out=w2_s, in_=w2d[:, :, :])
            nc.sync.dma_start(out=w3_s, in_=w3d[:, :, :])
            nc.sync.dma_start(out=M_f, in_=Md[:, :])
            nc.sync.dma_start(out=Nwc_s[:, 0], in_=Nwcd[:])
            for fc in range(4):
                nc.sync.dma_start(out=c1_s[:, fc], in_=c1d[fc, :])
            for mc in range(2):
                nc.sync.dma_start(out=c2_s[:, mc], in_=c2d[mc, :])
            nc.sync.dma_start(out=c3_s[:, 0], in_=c3d[:])
            nc.vector.tensor_copy(out=M_s, in_=M_f)

            # hx[f,i], hy''[f,j] = hy + c1 (bf16 setup matmuls)
            with tc.tile_pool(name="psum_setup", bufs=2, space="PSUM") as pp:
                for fc in range(4):
                    ph = pp.tile([128, N], F32, tag="ph")
                    nc.tensor.matmul(
                        ph, lhsT=wxT_s[:, fc * 128:(fc + 1) * 128],
                        rhs=xT_s, start=True, stop=True)
                    nc.scalar.activation(hx_s[:, fc, :], ph, AF.Copy)
                    py_ = pp.tile([128, N], F32, tag="ph")
                    nc.tensor.matmul(
                        py_, lhsT=wyT_s[:, fc * 128:(fc + 1) * 128],
                        rhs=yT_s, start=True, stop=True)
                    nc.scalar.activation(hy_s[:, fc, :], py_, AF.Identity,
                                         bias=c1_s[:, fc:fc + 1])
            for fc in range(4):
                nc.vector.tensor_copy(
                    out=hy_rep[:, fc, :, :],
                    in_=hy_s[:, fc, :].unsqueeze(2)
                        .broadcast_to([128, N, C]))

            work = ctx.enter_context(tc.tile_pool(name="work", bufs=2))
            psum = ctx.enter_context(tc.tile_pool(name="psum", bufs=2,
                                                  space="PSUM"))
            psum1 = ctx.enter_context(tc.tile_pool(name="psum1", bufs=1,
                                                   space="PSUM"))

            def emit_dot(ph3g, pv_sb, pg, tl):
                t = pg * G + tl
                pf4 = psum.tile([1, C * N], F32, tag="pf")
                for ii in range(C):
                    nc.tensor.matmul(
                        pf4[0:1, ii * N:(ii + 1) * N],
                        lhsT=pv_sb[:, tl * C + ii:tl * C + ii + 1],
                        rhs=ph3g[:, tl, ii * N:(ii + 1) * N],
                        start=True, stop=True)
                nc.scalar.activation(stage[0:1, t, :], pf4, AF.Copy)

            def emit_q_head(pq, ph3g, g):
                for tl in range(G):
                    with nc.allow_low_precision(reason="f32r q"):
                        nc.vector.tensor_reduce(
                            out=pq[:, tl * C:(tl + 1) * C],
                            in_=ph3g[:, tl, :].rearrange(
                                "p (a b) -> p a b", a=C),
                            axis=AX.X, op=ALU.add)
                pv = psum1.tile([128, G * C], F32, tag="pv")
                nc.tensor.matmul(pv, lhsT=M_s, rhs=pq, start=True, stop=True)
                v_sb = work.tile([128, G * C], BF16, tag="v")
                nc.scalar.activation(v_sb, pv, AF.Identity, bias=Nwc_s)
                nc.sync.dma_start(out=q_d[g, :, :], in_=pq.bitcast(F32))
                return v_sb

            prev = None      # (h3g, q_s, g) awaiting q/head/dots
            for g in range(NG):
                # ---- L1 for group g ----
                # (j-major, i-minor) pair order puts both broadcasts'
                # stride-0 off the last AP dim, keeping the DVE fast mode.
                h1g = work.tile([128, 4, G, C * N], BF16, tag="h1g")
                i0 = g * G * C
                for fc in range(4):
                    for tl in range(G):
                        ic = i0 + tl * C
                        nc.vector.scalar_tensor_tensor(
                            out=h1p[:, fc, tl, :].rearrange(
                                "p (b a) -> p b a", a=C),
                            in0=hx_s[:, fc, ic:ic + C].unsqueeze(1)
                                .broadcast_to([128, N, C]),
                            scalar=0.0,
                            in1=hy_rep[:, fc, :, :],
                            op0=ALU.add, op1=ALU.add)
                    nc.vector.tensor_scalar(
                        out=h1g[:, fc, :, :], in0=h1p[:, fc, :, :],
                        scalar1=0.0, scalar2=None, op0=ALU.max)

                # previous group: q + head (after L1 in DVE program order
                # so the DVE stream never stalls on the MLP pipeline)
                v_prev = None
                if prev is not None:
                    v_prev = emit_q_head(prev[1], prev[0], prev[2])

                h3g = work.tile([128, G, C * N], BF16, tag="h3g")
                q_s = work.tile([128, G * C], F32R, tag="q")

                for tl in range(G):
                    # ---- L2 ----
                    h2 = work.tile([128, 2, C * N], BF16, tag="h2")
                    for mc in range(2):
                        p2 = psum.tile([128, C * N], F32, tag="p2")
                        for kc in range(4):
                            nc.tensor.matmul(
                                p2,
                                lhsT=w2_s[:, kc, mc * 128:(mc + 1) * 128],
                                rhs=h1g[:, kc, tl, :],
                                start=(kc == 0), stop=(kc == 3))
                        nc.scalar.activation(h2[:, mc, :], p2, AF.Relu,
                                             bias=c2_s[:, mc:mc + 1])

                    if v_prev is not None:
                        emit_dot(prev[0], v_prev, prev[2], tl)

                    # ---- L3 ----
                    p3 = psum.tile([128, C * N], F32, tag="p3")
                    for kc in range(2):
                        nc.tensor.matmul(
                            p3, lhsT=w3_s[:, kc, :], rhs=h2[:, kc, :],
                            start=(kc == 0), stop=(kc == 1))
                    nc.scalar.activation(
                        h3g[:, tl, :].rearrange("p (a b) -> p b a", a=C),
                        p3, AF.Relu, bias=c3_s[:, 0:1])

                prev = (h3g, q_s, g)

            v_last = emit_q_head(prev[1], prev[0], prev[2])
            for tl in range(G):
                emit_dot(prev[0], v_last, prev[2], tl)

            nc.sync.dma_start(
                out=out_d[:, :].unsqueeze(0),
                in_=stage[0:1, :, :].rearrange("p a (c b) -> p (a c) b", c=C))
    nc.finalize()
    return nc


_NC_CACHE = None


def _get_nc():
    global _NC_CACHE
    if _NC_CACHE is None:
        _NC_CACHE = build_module()
    return _NC_CACHE


def host_prep(inputs):
    """Fold BN affines into weights/biases; pre-transpose into device
    layouts; cast to bf16. Returns (per-core input maps, wc, K0)."""
    f32 = np.float32
    x = np.asarray(inputs["x"], f32)
    y = np.asarray(inputs["y"], f32)
    w1, b1 = np.asarray(inputs["w1"], f32), np.asarray(inputs["b1"], f32)
    w2, b2 = np.asarray(inputs["w2"], f32), np.asarray(inputs["b2"], f32)
    w3, b3 = np.asarray(inputs["w3"], f32), np.asarray(inputs["b3"], f32)
    w4, b4 = np.asarray(inputs["w4"], f32), np.asarray(inputs["b4"], f32)

    k1 = inputs["g1"] / np.sqrt(inputs["v1"] + EPS)
    c1 = k1 * (b1 - inputs["m1"]) + inputs["be1"]
    k2 = inputs["g2"] / np.sqrt(inputs["v2"] + EPS)
    c2 = k2 * (b2 - inputs["m2"]) + inputs["be2"]
    k3 = inputs["g3"] / np.sqrt(inputs["v3"] + EPS)
    c3 = k3 * (b3 - inputs["m3"]) + inputs["be3"]

    wx = w1[:, :F] * k1[:, None]          # (512, 128)
    wy = w1[:, F:] * k1[:, None]
    w2f = w2 * k2[:, None]                # (256, 512)
    w3f = w3 * k3[:, None]                # (128, 256)

    # [k, kc, mc*128+m] = w2f[mc*128+m, kc*128+k]
    w2b = np.ascontiguousarray(
        w2f.T.reshape(4, 128, D2).transpose(1, 0, 2)).astype(NPBF)
    w3b = np.ascontiguousarray(
        w3f.T.reshape(2, 128, D3).transpose(1, 0, 2)).astype(NPBF)

    M = (w4.T @ w4).astype(f32)           # (128, 128), symmetric
    wc = (w4.T @ b4).astype(f32)          # (128,)
    K0 = float(N * (b4 @ b4))

    shared = {
        "wxT": np.ascontiguousarray(wx.T).astype(NPBF),
        "wyT": np.ascontiguousarray(wy.T).astype(NPBF),
        "w2b": w2b,
        "w3b": w3b,
        "M": np.ascontiguousarray(M),
        "Nwc": np.ascontiguousarray(N * wc, f32),
        "c1": np.ascontiguousarray(c1.reshape(4, 128), f32),
        "c2": np.ascontiguousarray(c2.reshape(2, 128), f32),
        "c3": np.ascontiguousarray(c3, f32),
    }
    in_maps = []
    for b in range(B):
        m = dict(shared)
        m["xT"] = np.ascontiguousarray(x[b].T).astype(NPBF)
        m["yT"] = np.ascontiguousarray(y[b].T).astype(NPBF)
        in_maps.append(m)
    return in_maps, wc, K0


def assemble(res, wc, K0):
    outs = []
    for b in range(B):
        o = res.results[b]["out"].astype(np.float32)          # (N, N)
        q = res.results[b]["q"].astype(np.float32)            # (NG,128,G*C)
        qf = np.concatenate([q[g] for g in range(NG)], axis=1)  # (128, N)
        c = qf.T @ wc + K0                                    # (N,)
        outs.append(o + c[:, None])
    return np.stack(outs, axis=0).astype(np.float32)


def kernel(**inputs):
    nc = _get_nc()
    in_maps, wc, K0 = host_prep(inputs)
    res = run_bass_kernel_spmd(nc, in_maps, list(range(B)))
    return assemble(res, wc, K0)


# revision 23
# speedup vs baseline: 1.2859x; 1.2859x over previous
"""Trainium2 Bass kernel for nn_DKOKernel (dense pairwise MLP + PSD head).

Math (per batch b, one NeuronCore per batch element):
  hx[f,i] = wx x_i;  hy[f,j] = wy y_j   (BN1 folded into wx/wy/c1)
  h1 = relu(hx_i + hy_j + c1)           (512)
  h2 = relu(W2 h1 + c2)                 (256)
  h3 = relu(W3 h2 + c3)                 (128)
  e  = W4 h3 + b4                       (64)
  s_i = sum_j e_ij;  out[i,j] = e_ij . s_i

Head algebra (never materializes e or s):
  q_i = sum_j h3_ij
  v_i = (W4^T W4) q_i + ny (W4^T b4) = M q_i + ny wc
  out[i,j] = h3_ij . v_i + c_i,  c_i = wc . q_i + ny|b4|^2
The +c_i term is applied on the HOST (q is shipped back), so the device
only computes the h3.v dot.

All matmuls run in bf16 (1 cycle/row); fp8 was measured too inaccurate
for the 2e-2 gate (h1-only fp8 already gives 1.7e-2).

Layout: features on partitions, (i-block, j) pairs on the free dim.
C=4 i-rows per chunk -> free dim 512 (fp32 PSUM bank max). Chunks are
processed in groups of G=4; L1 is produced per-group with two big DVE
ops per fc (broadcast add + relu) instead of 16 small fused ops.
"""

import os
import numpy as np
import ml_dtypes
from contextlib import ExitStack

import concourse.bacc as bacc
import concourse.tile as tile
from concourse import mybir
from concourse.bass_utils import run_bass_kernel_spmd

F32 = mybir.dt.float32
F32R = mybir.dt.float32r
BF16 = mybir.dt.bfloat16
AF = mybir.ActivationFunctionType
ALU = mybir.AluOpType
AX = mybir.AxisListType

EPS = 1e-5
B = 8
N = 128          # nx == ny
F = 128          # input feature dim
D1, D2, D3 = 512, 256, 128
C = 4            # i-rows per chunk -> 512 pairs per chunk
NCH = N // C     # 32 chunks
G = int(os.environ.get('GROUP', '4'))    # chunks per group
NPOOL_FC = int(os.environ.get('NPOOL_FC', '2'))  # fc groups whose L1 adds run on Pool
NG = NCH // G

NPBF = ml_dtypes.bfloat16


def build_module():
    nc = bacc.Bacc()

    xT = nc.declare_dram_parameter("xT", [F, N], BF16, isOutput=False)
    yT = nc.declare_dram_parameter("yT", [F, N], BF16, isOutput=False)
    wxT = nc.declare_dram_parameter("wxT", [F, D1], BF16, isOutput=False)
    wyT = nc.declare_dram_parameter("wyT", [F, D1], BF16, isOutput=False)
    w2d = nc.declare_dram_parameter("w2b", [128, 4, D2], BF16, isOutput=False)
    w3d = nc.declare_dram_parameter("w3b", [128, 2, D3], BF16, isOutput=False)
    Md = nc.declare_dram_parameter("M", [128, 128], F32, isOutput=False)
    Nwcd = nc.declare_dram_parameter("Nwc", [128], F32, isOutput=False)
    c1d = nc.declare_dram_parameter("c1", [4, 128], F32, isOutput=False)
    c2d = nc.declare_dram_parameter("c2", [2, 128], F32, isOutput=False)
    c3d = nc.declare_dram_parameter("c3", [128], F32, isOutput=False)
    out_d = nc.declare_dram_parameter("out", [N, N], BF16, isOutput=True)
    q_d = nc.declare_dram_parameter("q", [NG, 128, G * C], F32, isOutput=True)

    with tile.TileContext(nc) as tc:
        with ExitStack() as ctx:
            singles = ctx.enter_context(tc.tile_pool(name="singles", bufs=1))

            xT_s = singles.tile([F, N], BF16)
            yT_s = singles.tile([F, N], BF16)
            wxT_s = singles.tile([F, D1], BF16)
            wyT_s = singles.tile([F, D1], BF16)
            w2_s = singles.tile([128, 4, D2], BF16)
            w3_s = singles.tile([128, 2, D3], BF16)
            M_f = singles.tile([128, 128], F32)
            M_s = singles.tile([128, 128], F32R)
            Nwc_s = singles.tile([128, 1], F32)
            c1_s = singles.tile([128, 4], F32)
            c2_s = singles.tile([128, 2], F32)
            c3_s = singles.tile([128, 1], F32)
            hx_s = singles.tile([128, 4, N], BF16)
            hy_s = singles.tile([128, 4, N], BF16)
            stage = singles.tile([1, NCH, C * N], BF16)
            h1p = singles.tile([128, 4, G, C * N], BF16)

            nc.sync.dma_start(out=xT_s, in_=xT[:, :])
            nc.sync.dma_start(out=yT_s, in_=yT[:, :])
            nc.sync.dma_start(out=wxT_s, in_=wxT[:, :])
            nc.sync.dma_start(out=wyT_s, in_=wyT[:, :])
            nc.sync.dma_start(out=w2_s, in_=w2d[:, :, :])
            nc.sync.dma_start(out=w3_s, in_=w3d[:, :, :])
            nc.sync.dma_start(out=M_f, in_=Md[:, :])
            nc.sync.dma_start(out=Nwc_s[:, 0], in_=Nwcd[:])
            for fc in range(4):
                nc.sync.dma_start(out=c1_s[:, fc], in_=c1d[fc, :])
            for mc in range(2):
                nc.sync.dma_start(out=c2_s[:, mc], in_=c2d[mc, :])
            nc.sync.dma_start(out=c3_s[:, 0], in_=c3d[:])
            nc.vector.tensor_copy(out=M_s, in_=M_f)

            # hx[f,i], hy''[f,j] = hy + c1 (bf16 setup matmuls)
            with tc.tile_pool(name="psum_setup", bufs=2, space="PSUM") as pp:
                for fc in range(4):
                    ph = pp.tile([128, N], F32, tag="ph")
                    nc.tensor.matmul(
                        ph, lhsT=wxT_s[:, fc * 128:(fc + 1) * 128],
                        rhs=xT_s, start=True, stop=True)
                    nc.scalar.activation(hx_s[:, fc, :], ph, AF.Copy)
                    py_ = pp.tile([128, N], F32, tag="ph")
                    nc.tensor.matmul(
                        py_, lhsT=wyT_s[:, fc * 128:(fc + 1) * 128],
                        rhs=yT_s, start=True, stop=True)
                    nc.scalar.activation(hy_s[:, fc, :], py_, AF.Identity,
                                         bias=c1_s[:, fc:fc + 1])

            work = ctx.enter_context(tc.tile_pool(name="work", bufs=2))
            psum = ctx.enter_context(tc.tile_pool(name="psum", bufs=2,
                                                  space="PSUM"))
            psum1 = ctx.enter_context(tc.tile_pool(name="psum1", bufs=1,
                                                   space="PSUM"))

            def emit_dot(ph3g, pv_sb, pg, tl):
                t = pg * G + tl
                pf4 = psum.tile([1, C * N], F32, tag="pf")
                for ii in range(C):
                    nc.tensor.matmul(
                        pf4[0:1, ii * N:(ii + 1) * N],
                        lhsT=pv_sb[:, tl * C + ii:tl * C + ii + 1],
                        rhs=ph3g[:, tl, ii * N:(ii + 1) * N],
                        start=True, stop=True)
                nc.scalar.activation(stage[0:1, t, :], pf4, AF.Copy)

            def emit_q_head(pq, ph3g, g):
                for tl in range(G):
                    with nc.allow_low_precision(reason="f32r q"):
                        nc.vector.tensor_reduce(
                            out=pq[:, tl * C:(tl + 1) * C],
                            in_=ph3g[:, tl, :].rearrange(
                                "p (a b) -> p a b", a=C),
                            axis=AX.X, op=ALU.add)
                pv = psum1.tile([128, G * C], F32, tag="pv")
                nc.tensor.matmul(pv, lhsT=M_s, rhs=pq, start=True, stop=True)
                v_sb = work.tile([128, G * C], BF16, tag="v")
                nc.scalar.activation(v_sb, pv, AF.Identity, bias=Nwc_s)
                nc.sync.dma_start(out=q_d[g, :, :], in_=pq.bitcast(F32))
                return v_sb

            prev = None      # (h3g, q_s, g) awaiting q/head/dots
            for g in range(NG):
                # ---- L1 for group g ----
                # (j-major, i-minor) pair order puts both broadcasts'
                # stride-0 off the last AP dim, keeping the DVE fast mode.
                h1g = work.tile([128, 4, G, C * N], BF16, tag="h1g")
                i0 = g * G * C
                for fc in range(4):
                    if fc < NPOOL_FC:
                        # per-chunk broadcast adds on the (otherwise idle)
                        # Pool engine
                        for tl in range(G):
                            ic = i0 + tl * C
                            nc.gpsimd.tensor_tensor(
                                out=h1p[:, fc, tl, :].rearrange(
                                    "p (a b) -> p a b", a=C),
                                in0=hy_s[:, fc, :].unsqueeze(1)
                                    .broadcast_to([128, C, N]),
                                in1=hx_s[:, fc, ic:ic + C].unsqueeze(2)
                                    .broadcast_to([128, C, N]),
                                op=ALU.add)
                    else:
                        nc.vector.scalar_tensor_tensor(
                            out=h1p[:, fc, :, :].rearrange(
                                "p a (c b) -> p (a c) b", c=C),
                            in0=hx_s[:, fc, i0:i0 + G * C].unsqueeze(2)
                                .broadcast_to([128, G * C, N]),
                            scalar=0.0,
                            in1=hy_s[:, fc, :].unsqueeze(1)
                                .broadcast_to([128, G * C, N]),
                            op0=ALU.add, op1=ALU.add)
                    nc.vector.tensor_scalar(
                        out=h1g[:, fc, :, :], in0=h1p[:, fc, :, :],
                        scalar1=0.0, scalar2=None, op0=ALU.max)

                # previous group: q + head (after L1 in DVE program order
                # so the DVE stream never stalls on the MLP pipeline)
                v_prev = None
                if prev is not None:
                    v_prev = emit_q_head(prev[1], prev[0], prev[2])

                h3g = work.tile([128, G, C * N], BF16, tag="h3g")
                q_s = work.tile([128, G * C], F32R, tag="q")

                for tl in range(G):
                    # ---- L2 ----
                    h2 = work.tile([128, 2, C * N], BF16, tag="h2")
                    for mc in range(2):
                        p2 = psum.tile([128, C * N], F32, tag="p2")
                        for kc in range(4):
                            nc.tensor.matmul(
                                p2,
                                lhsT=w2_s[:, kc, mc * 128:(mc + 1) * 128],
                                rhs=h1g[:, kc, tl, :],
                                start=(kc == 0), stop=(kc == 3))
                        nc.scalar.activation(h2[:, mc, :], p2, AF.Relu,
                                             bias=c2_s[:, mc:mc + 1])

                    if v_prev is not None:
                        emit_dot(prev[0], v_prev, prev[2], tl)

                    # ---- L3 ----
                    p3 = psum.tile([128, C * N], F32, tag="p3")
                    for kc in range(2):
                        nc.tensor.matmul(
                            p3, lhsT=w3_s[:, kc, :], rhs=h2[:, kc, :],
                            start=(kc == 0), stop=(kc == 1))
                    nc.scalar.activation(h3g[:, tl, :], p3, AF.Relu,
                                         bias=c3_s[:, 0:1])

                prev = (h3g, q_s, g)

            v_last = emit_q_head(prev[1], prev[0], prev[2])
            for tl in range(G):
                emit_dot(prev[0], v_last, prev[2], tl)

            nc.sync.dma_start(
                out=out_d[:, :].unsqueeze(0),
                in_=stage[0:1, :, :].rearrange("p a (c b) -> p (a c) b", c=C))
    nc.finalize()
    return nc


_NC_CACHE = None


def _get_nc():
    global _NC_CACHE
    if _NC_CACHE is None:
        _NC_CACHE = build_module()
    return _NC_CACHE


def host_prep(inputs):
    """Fold BN affines into weights/biases; pre-transpose into device
    layouts; cast to bf16. Returns (per-core input maps, wc, K0)."""
    f32 = np.float32
    x = np.asarray(inputs["x"], f32)
    y = np.asarray(inputs["y"], f32)
    w1, b1 = np.asarray(inputs["w1"], f32), np.asarray(inputs["b1"], f32)
    w2, b2 = np.asarray(inputs["w2"], f32), np.asarray(inputs["b2"], f32)
    w3, b3 = np.asarray(inputs["w3"], f32), np.asarray(inputs["b3"], f32)
    w4, b4 = np.asarray(inputs["w4"], f32), np.asarray(inputs["b4"], f32)

    k1 = inputs["g1"] / np.sqrt(inputs["v1"] + EPS)
    c1 = k1 * (b1 - inputs["m1"]) + inputs["be1"]
    k2 = inputs["g2"] / np.sqrt(inputs["v2"] + EPS)
    c2 = k2 * (b2 - inputs["m2"]) + inputs["be2"]
    k3 = inputs["g3"] / np.sqrt(inputs["v3"] + EPS)
    c3 = k3 * (b3 - inputs["m3"]) + inputs["be3"]

    wx = w1[:, :F] * k1[:, None]          # (512, 128)
    wy = w1[:, F:] * k1[:, None]
    w2f = w2 * k2[:, None]                # (256, 512)
    w3f = w3 * k3[:, None]                # (128, 256)

    # [k, kc, mc*128+m] = w2f[mc*128+m, kc*128+k]
    w2b = np.ascontiguousarray(
        w2f.T.reshape(4, 128, D2).transpose(1, 0, 2)).astype(NPBF)
    w3b = np.ascontiguousarray(
        w3f.T.reshape(2, 128, D3).transpose(1, 0, 2)).astype(NPBF)

    M = (w4.T @ w4).astype(f32)           # (128, 128), symmetric
    wc = (w4.T @ b4).astype(f32)          # (128,)
    K0 = float(N * (b4 @ b4))

    shared = {
        "wxT": np.ascontiguousarray(wx.T).astype(NPBF),
        "wyT": np.ascontiguousarray(wy.T).astype(NPBF),
        "w2b": w2b,
        "w3b": w3b,
        "M": np.ascontiguousarray(M),
        "Nwc": np.ascontiguousarray(N * wc, f32),
        "c1": np.ascontiguousarray(c1.reshape(4, 128), f32),
        "c2": np.ascontiguousarray(c2.reshape(2, 128), f32),
        "c3": np.ascontiguousarray(c3, f32),
    }
    in_maps = []
    for b in range(B):
        m = dict(shared)
        m["xT"] = np.ascontiguousarray(x[b].T).astype(NPBF)
        m["yT"] = np.ascontiguousarray(y[b].T).astype(NPBF)
        in_maps.append(m)
    return in_maps, wc, K0


def assemble(res, wc, K0):
    outs = []
    for b in range(B):
        o = res.results[b]["out"].astype(np.float32)          # (N, N)
        q = res.results[b]["q"].astype(np.float32)            # (NG,128,G*C)
        qf = np.concatenate([q[g] for g in range(NG)], axis=1)  # (128, N)
        c = qf.T @ wc + K0                                    # (N,)
        outs.append(o + c[:, None])
    return np.stack(outs, axis=0).astype(np.float32)


def kernel(**inputs):
    nc = _get_nc()
    in_maps, wc, K0 = host_prep(inputs)
    res = run_bass_kernel_spmd(nc, in_maps, list(range(B)))
    return assemble(res, wc, K0)
